# revision 1
# baseline (speedup 1.0000x reference)
"""ComplexRNN Trainium2 kernel.

10-layer tanh RNN, B=1024, T=512, D=16, H=30, final FC on last timestep.

Strategy (per core, 8-way batch-parallel, 128 batch rows/core):
  - Hidden-major layout: state h^l lives in SBUF as [30 partitions, 128 batch].
  - Layer wavefront: at step s, layer l computes timestep t = s - l. All
    10 layers advance each step; all dependencies are on step s-1.
  - States packed into 3 "region" windows of 128 partitions (4 slots of 32):
      R0 = [h0 h1 h2 h3], R1 = [h3' h4 h5 h6], R2 = [h6' h7 h8 h9]
    (h3', h6' are duplicates written by extra matmuls so each layer finds
    its feed + recurrent state inside one 128-partition window).
  - ONE fp16 matmul per region per step (K=128, M=128, N=128): a block
    matrix computes all of the region's layer updates at once, with the
    combined biases on a "ones row" (partition 126 of each window). The
    ones row is self-regenerating: output column 126 maps ones -> 30.0
    -> tanh -> 1.0. Plus one "inject" matmul for x_t @ W_ih0^T (reading
    a host-pre-transposed x ring) and two duplicate matmuls (h3', h6'),
    whose weights are column slices of the merged region matrices.
  - tanh: ACTIVATE (PSUM -> SBUF fp16); ScalarE is the critical engine.
  - 3-deep rotation of state buffers so the only cross-step dependencies
    are true RAW deps.
"""

import copy
import numpy as np

import concourse.bass as bass
import concourse.tile as tile
from concourse import mybir
from concourse import bass_utils

N_CORES = 8
B, T, D, H, L = 1024, 512, 16, 30, 10
BC = B // N_CORES          # batch per core = 128
RING = 8                   # resident x blocks
ACT_SPLIT = 2              # 1: one ACTIVATE/step, 2: split R0+R1 / R2
DEPTH = 3                  # state rotation depth

F16 = mybir.dt.float16
F32 = mybir.dt.float32

# wbuf column layout (all matmul weight blocks are 128 cols so every
# matmul is full 128x128 mode -- no PE tiling-mode switches)
WC_R0, WC_R1, WC_R2 = 0, 128, 256
WC_INJ = 384               # + 128*v, v=0..7
WC_DUP3 = WC_INJ + 8 * 128
WC_DUP6 = WC_DUP3 + 128
WC_FC = WC_DUP6 + 128
W_COLS = WC_FC + 32


def _split_sync_waits(nc, limit=1):
    """walrus CoreV2/V3 lowering rejects instructions whose sync_info carries
    more than ~1 wait condition. Hoist excess waits onto same-engine NoOps
    inserted immediately before the offending instruction (engines execute
    their stream in order, so the waits still gate it)."""
    for fn in nc.m.functions:
        for blk in fn.blocks:
            newlist = []
            for inst in blk.instructions:
                si = inst.sync_info
                if si is not None and si.on_wait and len(si.on_wait) > limit:
                    waits = list(si.on_wait)
                    extra, keep = waits[:-limit], waits[-limit:]
                    for j, w in enumerate(extra):
                        pre = mybir.InstNoOp(
                            name=f"{inst.name}_w{j}",
                            sync_info=mybir.SyncInfo(on_wait=[w], on_update=[]),
                            bass_nofuse=True,
                            engine=inst.engine,
                        )
                        nc.register_instruction(pre, overwrite=True)
                        newlist.append(pre)
                    inst.sync_info = copy.replace(si, on_wait=keep)
                newlist.append(inst)
            blk.instructions = newlist


def build_kernel(t_steps=T):
    nblk = (t_steps + 7) // 8
    xt_blocks = nblk + 8
    n_steps = t_steps + L - 1  # wavefront steps

    nc = bass.Bass(trn_type="TRN2")
    xt = nc.dram_tensor("xt", [xt_blocks * 128, BC], F16, kind="ExternalInput")
    sinit = nc.dram_tensor("sinit", [128, 2 * BC], F16, kind="ExternalInput")
    wbuf = nc.dram_tensor("wbuf", [128, W_COLS], F16, kind="ExternalInput")
    y = nc.dram_tensor("y", [1, BC], F32, kind="ExternalOutput")

    with tile.TileContext(nc) as tc:
        with (
            tc.tile_pool(name="persist", bufs=1) as pp,
            tc.tile_pool(name="psum", bufs=1, space="PSUM") as pq,
        ):
            wt = pp.tile([128, W_COLS], F16, tag="wt", name="wt")
            ring = [pp.tile([128, BC], F16, tag=f"ring{i}", name=f"ring{i}")
                    for i in range(RING)]
            sa = [pp.tile([128, 2 * BC], F16, tag=f"sa{i}", name=f"sa{i}")
                  for i in range(DEPTH)]
            sb = [pp.tile([128, BC], F16, tag=f"sb{i}", name=f"sb{i}")
                  for i in range(DEPTH)]
            if ACT_SPLIT == 1:
                sab = [pp.tile([128, 3 * BC], F16, tag=f"sab{i}",
                               name=f"sab{i}") for i in range(DEPTH)]
                sa = [t_[:, 0:2 * BC] for t_ in sab]
                sb = [t_[:, 2 * BC:3 * BC] for t_ in sab]
                pc = [pq.tile([128, 3 * BC], F32, tag=f"pc{i}", name=f"pc{i}")
                      for i in range(3)]
                pa = [t_[:, 0:2 * BC] for t_ in pc]
                pb = [t_[:, 2 * BC:3 * BC] for t_ in pc]
                PDEPTH = 3
            else:
                pa = [pq.tile([128, 2 * BC], F32, tag=f"pa{i}", name=f"pa{i}")
                      for i in range(3)]
                pb = [pq.tile([128, BC], F32, tag=f"pb{i}", name=f"pb{i}")
                      for i in range(3)]
                PDEPTH = 3
            pfc = pq.tile([1, BC], F32, tag="pfc", name="pfc")

            # --- init ---
            nc.sync.dma_start(out=wt[:, :], in_=wbuf[:, :])
            for i in range(RING):
                nc.sync.dma_start(out=ring[i][:, :],
                                  in_=xt[i * 128:(i + 1) * 128, :])
            for p_ in pa:
                nc.vector.memset(p_[:, :], 0.0)
            for p_ in pb:
                nc.vector.memset(p_[:, :], 0.0)
            nc.vector.memset(pfc[:, :], 0.0)
            for s_ in sa:
                nc.sync.dma_start(out=s_[:, :], in_=sinit[:, :])
            for s_ in sb:
                nc.sync.dma_start(out=s_[:, :], in_=sinit[:, 0:BC])

            def emit_step(s):
                j = (s - 1) % DEPTH  # state buffers of step s-1
                k = s % DEPTH
                kp = s % PDEPTH
                r0 = sa[j][:, 0:BC]
                r1 = sa[j][:, BC:2 * BC]
                r2 = sb[j][:, :]

                if s % 8 == 0:
                    b = s // 8 + 4
                    if b < xt_blocks:
                        nc.sync.dma_start(out=ring[b % RING][:, :],
                                          in_=xt[b * 128:(b + 1) * 128, :])

                inj = s < t_steps
                # R0 merged + inject
                nc.tensor.matmul(pa[kp][0:128, 0:BC], wt[:, WC_R0:WC_R0 + 128],
                                 r0, start=True, stop=not inj,
                                 skip_group_check=True)
                if inj:
                    blk = (s // 8) % RING
                    nc.tensor.matmul(pa[kp][0:128, 0:BC],
                                     wt[:, WC_INJ + 128 * (s % 8):
                                        WC_INJ + 128 * (s % 8) + 128],
                                     ring[blk][:, :], start=False, stop=True,
                                     skip_group_check=True)
                # R1 merged + dup3 (dup3 weights = cols 96:126 of R0 matrix)
                nc.tensor.matmul(pa[kp][0:128, BC:2 * BC],
                                 wt[:, WC_R1:WC_R1 + 128],
                                 r1, start=True, stop=False,
                                 skip_group_check=True)
                nc.tensor.matmul(pa[kp][0:128, BC:2 * BC],
                                 wt[:, WC_DUP3:WC_DUP3 + 128],
                                 r0, start=False, stop=True,
                                 skip_group_check=True)
                # R2 merged + dup6
                nc.tensor.matmul(pb[kp][0:128, 0:BC], wt[:, WC_R2:WC_R2 + 128],
                                 r2, start=True, stop=False,
                                 skip_group_check=True)
                nc.tensor.matmul(pb[kp][0:128, 0:BC],
                                 wt[:, WC_DUP6:WC_DUP6 + 128],
                                 r1, start=False, stop=True,
                                 skip_group_check=True)

                # ---- activations ----
                if ACT_SPLIT == 1:
                    nc.scalar.activation(sab[k][:, :], pc[kp][:, :],
                                         mybir.ActivationFunctionType.Tanh)
                else:
                    nc.scalar.activation(sa[k][:, :], pa[kp][:, :],
                                         mybir.ActivationFunctionType.Tanh)
                    nc.scalar.activation(sb[k][:, :], pb[kp][:, :],
                                         mybir.ActivationFunctionType.Tanh)

                # ---- warmup zeroing: slot h^(s+1) must be 0 before step s+1
                l = s + 1
                if 1 <= l <= 9:
                    tgt = [None,
                           (0, 32, 62, 0), (0, 64, 94, 0), (0, 96, 126, 0),
                           (0, 32, 62, 1), (0, 64, 94, 1), (0, 96, 126, 1),
                           (1, 32, 62, 0), (1, 64, 94, 0), (1, 96, 126, 0),
                           ][l]
                    which, p0, p1, half = tgt
                    if which == 0:
                        nc.sync.dma_start(
                            out=sa[k][p0:p1, half * BC:(half + 1) * BC],
                            in_=sinit[0:p1 - p0, 0:BC])
                    else:
                        nc.sync.dma_start(out=sb[k][p0:p1, :],
                                          in_=sinit[0:p1 - p0, 0:BC])

            for s in range(n_steps):
                emit_step(s)

            # ---- FC on h9 of last timestep (in sb[(n_steps-1)%DEPTH] slot 3)
            fin = sb[(n_steps - 1) % DEPTH][:, :]
            nc.tensor.matmul(pfc[0:1, :], wt[:, WC_FC:WC_FC + 1], fin,
                             start=True, stop=True)
            yout = pp.tile([1, BC], F32, tag="yout", name="yout")
            nc.vector.tensor_copy(yout[0:1, :], pfc[0:1, :])
            nc.sync.dma_start(out=y[:, :], in_=yout[0:1, :])

    _split_sync_waits(nc)
    return nc


def prep_core_inputs(x_core, W_ih0, W_ih, W_hh, b_ih, b_hh, fc_w, fc_b,
                     t_steps=T):
    """Host-side marshaling for one core. x_core: [BC, t_steps, D] fp32."""
    nblk = (t_steps + 7) // 8
    xt_blocks = nblk + 8
    # XT[g*128 + 16*(t%8)+d, b] = x[b, 8g + t%8, d]
    xt = np.zeros((xt_blocks * 128, BC), np.float16)
    xr = np.transpose(x_core, (1, 2, 0))  # [t, d, b]
    tpad = nblk * 8
    if t_steps != tpad:
        xr = np.concatenate([xr, np.zeros((tpad - t_steps, D, BC), xr.dtype)],
                            0)
    xt[:nblk * 128, :] = xr.reshape(nblk * 128, BC)

    wbuf = np.zeros((128, W_COLS), np.float32)

    def put_region(col0, layers):
        # layers: (out_slot, feed_slot_or_None, Wi_or_None, rec_slot, Wh, bias)
        for out_slot, fslot, Wi, rslot, Wh, bias in layers:
            c = col0 + 32 * out_slot
            if Wi is not None:
                wbuf[32 * fslot:32 * fslot + Wi.shape[1], c:c + 30] = Wi.T
            wbuf[32 * rslot:32 * rslot + 30, c:c + 30] = Wh.T
            wbuf[126, c:c + 30] = bias
        wbuf[126, col0 + 126] = 30.0  # ones-row regeneration

    bias = b_ih + b_hh
    put_region(WC_R0, [
        (0, None, None, 0, W_hh[0], bias[0]),
        (1, 0, W_ih[0], 1, W_hh[1], bias[1]),
        (2, 1, W_ih[1], 2, W_hh[2], bias[2]),
        (3, 2, W_ih[2], 3, W_hh[3], bias[3]),
    ])
    put_region(WC_R1, [
        (1, 0, W_ih[3], 1, W_hh[4], bias[4]),
        (2, 1, W_ih[4], 2, W_hh[5], bias[5]),
        (3, 2, W_ih[5], 3, W_hh[6], bias[6]),
    ])
    put_region(WC_R2, [
        (1, 0, W_ih[6], 1, W_hh[7], bias[7]),
        (2, 1, W_ih[7], 2, W_hh[8], bias[8]),
        (3, 2, W_ih[8], 3, W_hh[9], bias[9]),
    ])
    for v in range(8):
        wbuf[16 * v:16 * v + 16, WC_INJ + 128 * v:WC_INJ + 128 * v + 30] = \
            W_ih0.T
    # dup3 = layer-3 columns of R0 matrix; dup6 = layer-6 columns of R1
    wbuf[:, WC_DUP3:WC_DUP3 + 30] = wbuf[:, WC_R0 + 96:WC_R0 + 126]
    wbuf[:, WC_DUP6:WC_DUP6 + 30] = wbuf[:, WC_R1 + 96:WC_R1 + 126]
    wbuf[96:126, WC_FC] = fc_w[0]
    wbuf[126, WC_FC] = fc_b[0]

    sinit = np.zeros((128, 2 * BC), np.float16)
    sinit[126, :] = 1.0
    return {"xt": xt, "wbuf": wbuf.astype(np.float16), "sinit": sinit}


_CACHE = {}


def run(x, W_ih0, W_ih, W_hh, b_ih, b_hh, fc_w, fc_b, t_steps=T):
    x = np.asarray(x, np.float32)
    args = [np.asarray(a, np.float32) for a in
            (W_ih0, W_ih, W_hh, b_ih, b_hh, fc_w, fc_b)]
    key = t_steps
    if key not in _CACHE:
        _CACHE[key] = build_kernel(t_steps)
    nc = _CACHE[key]
    in_maps = [prep_core_inputs(x[c * BC:(c + 1) * BC], *args, t_steps=t_steps)
               for c in range(N_CORES)]
    res = bass_utils.run_bass_kernel_spmd(nc, in_maps,
                                          core_ids=list(range(N_CORES)))
    out = np.concatenate([res.results[c]["y"].reshape(BC, 1)
                          for c in range(N_CORES)], axis=0)
    return out, res


def kernel(x, W_ih0, W_ih, W_hh, b_ih, b_hh, fc_w, fc_b):
    out, _ = run(x, W_ih0, W_ih, W_hh, b_ih, b_hh, fc_w, fc_b)
    return out


if __name__ == "__main__":
    t_small = 32
    rng = np.random.default_rng(0)
    s = 1.0 / np.sqrt(H)
    x = rng.standard_normal((B, t_small, D)).astype(np.float32)
    W_ih0 = (rng.standard_normal((H, D)) * s).astype(np.float32)
    W_ih = (rng.standard_normal((L - 1, H, H)) * s).astype(np.float32)
    W_hh = (rng.standard_normal((L, H, H)) * s).astype(np.float32)
    b_ih = (rng.standard_normal((L, H)) * s).astype(np.float32)
    b_hh = (rng.standard_normal((L, H)) * s).astype(np.float32)
    fc_w = (rng.standard_normal((1, H)) * s).astype(np.float32)
    fc_b = (rng.standard_normal((1,)) * s).astype(np.float32)

    def ref_np(x):
        out = x
        for l in range(L):
            Wi = W_ih0 if l == 0 else W_ih[l - 1]
            xw = np.einsum("btd,hd->bth", out, Wi) + (b_ih[l] + b_hh[l])
            h = np.zeros((x.shape[0], H), np.float32)
            ys = np.empty((x.shape[0], xw.shape[1], H), np.float32)
            for t in range(xw.shape[1]):
                h = np.tanh(xw[:, t] + h @ W_hh[l].T)
                ys[:, t] = h
            out = ys
        return out[:, -1, :] @ fc_w.T + fc_b

    want = ref_np(x)
    got, _ = run(x, W_ih0, W_ih, W_hh, b_ih, b_hh, fc_w, fc_b, t_steps=t_small)
    err = np.abs(got - want).max() / (np.abs(want).max() + 1e-9)
    print("small-T rel err:", err)

